# revision 23
# baseline (speedup 1.0000x reference)
"""Trainium2 Bass kernel for MllamaTextSdpaAttention (GQA + RoPE + causal SDPA).

Strategy: tensor-parallel over heads across 8 NeuronCores. Core c owns
q-heads [4c, 4c+4) and kv-head c (kv groups intact). Each core computes
hidden @ Wq/Wk/Wv slices, RoPE, causal attention for its heads, and its
row-slice of the Wo matmul, yielding a partial [T, DIM] output (bf16).
The host sums the 8 partials in f32.

Layout tricks:
- hidden_states is fed transposed ([DIM, T], bf16) so every projection
  matmul has the contraction dim (features) on partitions.
- Q/K projections produce Q^T/K^T directly (head_dim=128 on partitions).
- The RoPE even/odd pairing is de-interleaved by permuting Wq/Wk columns
  on the host, turning RoPE into a half-rotation: the partner element sits
  64 partitions away, reachable with plain partition-offset slices. The
  d-permutation cancels in q.k^T. The 1/sqrt(d) scale is folded into Q's
  cos/sin tables.
- Scores are computed TRANSPOSED: scT[k, q] = K_rot^T(tile).T @ Q_rot^T.
  exp(scT) is then directly the moving operand for the P@V matmul
  (out^T[d,q] = V[k,d].T @ expT[k,q]) -- no P transposes or PSUM->SBUF
  P copies. Softmax denominators come from a ones-vector matmul on the
  PE (sum over k = partition dim), and the 1/sum normalization is applied
  to the small out^T tile (via a PE-broadcast of the reciprocal row),
  not to P. No max-subtraction: scores are bounded (|s| <= ~20) so f32
  exp is safe, and masked entries use the additive -1e9 mask -> exp = 0.
- Causality at 128-block granularity: k-blocks strictly above the
  diagonal are never computed or read; diagonal blocks get the transposed
  additive mask from the actual attention_mask input.
- The 1/rowsum reciprocal row is broadcast across partitions on the idle
  GpSimd engine (partition_broadcast), and each group's normalization
  epilogue is deferred into the next group (software pipelining) so the
  PE never waits on the DVE reciprocal.
- Emission interleaves projection chunks with the attention groups they
  unblock (chunk0 -> b0/qb0 groups -> chunk1 -> b0/qb1 groups -> ...) and
  all [128,512]-f32 PSUM scratch (projection accumulators, score tiles,
  output accumulators) shares one 5-slot pool (+2 ot +1 rs = 8 banks)
  so the whole kernel fits PSUM without phase barriers.
- TimelineSim (instruction cost model): ~353 us/core; PE busy ~326 us
  (92% occupancy), which is the bf16 matmul-column floor for this
  decomposition.
"""

import numpy as np
import ml_dtypes

import concourse.bacc as bacc
import concourse.bass as bass
import concourse.mybir as mybir
from concourse.tile import TileContext
from concourse import bass_utils

BF16 = mybir.dt.bfloat16
F32 = mybir.dt.float32

B, S, DIM = 2, 1024, 4096
T = B * S                     # 2048 tokens, batch-major
N_HEADS, N_KV = 32, 8
HD = 128                      # head dim == partition count
N_CORES = 8
HL = N_HEADS // N_CORES       # 4 local q-heads per core
KT = DIM // 128               # 32 feature tiles
CH = 512                      # projection token-chunk
NCHUNK = T // CH
QB = 512                      # attention q-block width
TT = T // 128                 # 16 token tiles global
SCALE = 1.0 / float(np.sqrt(HD))

_CACHE: dict = {}


def _build():
    nc = bacc.Bacc("TRN2", target_bir_lowering=False, debug=False,
                   enable_asserts=False)

    hsT = nc.dram_tensor("hsT", [DIM, T], BF16, kind="ExternalInput")
    wq = nc.dram_tensor("wq", [DIM, HL * HD], BF16, kind="ExternalInput")
    wk = nc.dram_tensor("wk", [DIM, HD], BF16, kind="ExternalInput")
    wv = nc.dram_tensor("wv", [DIM, HD], BF16, kind="ExternalInput")
    wo = nc.dram_tensor("wo", [HL * HD, DIM], BF16, kind="ExternalInput")
    cos_q = nc.dram_tensor("cos_q", [HD, T], BF16, kind="ExternalInput")
    sin_q = nc.dram_tensor("sin_q", [HD, T], BF16, kind="ExternalInput")
    cos_k = nc.dram_tensor("cos_k", [HD, T], BF16, kind="ExternalInput")
    sin_k = nc.dram_tensor("sin_k", [HD, T], BF16, kind="ExternalInput")
    maskT = nc.dram_tensor("maskT", [128, 128], F32, kind="ExternalInput")
    out = nc.dram_tensor("out", [T, DIM], BF16, kind="ExternalOutput")

    Exp = mybir.ActivationFunctionType.Exp

    with TileContext(nc) as tc:
        with tc.tile_pool(name="consts", bufs=1) as cpool, \
             tc.tile_pool(name="hs", bufs=2) as hpool, \
             tc.tile_pool(name="rope_tmp", bufs=2) as rpool, \
             tc.tile_pool(name="work_ps", bufs=5, space=bass.MemorySpace.PSUM) as wpool, \
             tc.tile_pool(name="ot_ps", bufs=2, space=bass.MemorySpace.PSUM) as otpool, \
             tc.tile_pool(name="rs_ps", bufs=1, space=bass.MemorySpace.PSUM) as rspool, \
             tc.tile_pool(name="et", bufs=6) as epool, \
             tc.tile_pool(name="bc_sb", bufs=2) as bcsbpool, \
             tc.tile_pool(name="recip", bufs=4) as rcpool, \
             tc.tile_pool(name="out_sb", bufs=4) as xsbpool:

            wq_h = [cpool.tile([128, KT, HD], BF16, tag=f"wq{m}", name=f"wq{m}")
                    for m in range(HL)]
            wk_sb = cpool.tile([128, KT, HD], BF16, tag="wk")
            wv_sb = cpool.tile([128, KT, HD], BF16, tag="wv")
            cq_sb = cpool.tile([128, T], BF16, tag="cq")
            sq_sb = cpool.tile([128, T], BF16, tag="sq")
            ck_sb = cpool.tile([128, T], BF16, tag="ck")
            sk_sb = cpool.tile([128, T], BF16, tag="sk")
            maskT_sb = cpool.tile([128, 128], F32, tag="maskT")
            ones_k = cpool.tile([128, 1], BF16, tag="ones_k")
            qt_rot = cpool.tile([128, HL, T], BF16, tag="qt")
            kt_rot = cpool.tile([128, T], BF16, tag="kt")
            v_sb = cpool.tile([128, TT, HD], BF16, tag="v")
            ao = cpool.tile([128, HL, T], BF16, tag="ao")

            wq_r = wq.ap().rearrange("(kt p) n -> p kt n", p=128)
            hsT_r = hsT.ap().rearrange("(kt p) t -> p kt t", p=128)

            # startup-critical DMA first: the k-tiles the first matmuls touch
            nc.sync.dma_start(wq_h[0][:, 0:8, :], wq_r[:, 0:8, 0:HD])
            nc.sync.dma_start(wq_h[0][:, 8:KT, :], wq_r[:, 8:KT, 0:HD])

            def late_consts():
                nc.sync.dma_start(wq_h[1], wq_r[:, :, HD:2 * HD])
                nc.sync.dma_start(cq_sb, cos_q.ap())
                nc.sync.dma_start(sq_sb, sin_q.ap())
                for m in range(2, HL):
                    nc.sync.dma_start(wq_h[m], wq_r[:, :, m * HD:(m + 1) * HD])
                nc.sync.dma_start(wk_sb, wk.ap().rearrange("(kt p) n -> p kt n", p=128))
                nc.sync.dma_start(ck_sb, cos_k.ap())
                nc.sync.dma_start(sk_sb, sin_k.ap())
                nc.sync.dma_start(wv_sb, wv.ap().rearrange("(kt p) n -> p kt n", p=128))
                nc.sync.dma_start(maskT_sb, maskT.ap())
                nc.vector.memset(ones_k, 1.0)

            def rope(ps, out_ap, cos_ap, sin_ap):
                """out = ps*cos + halfswap(ps)*sin  (signs baked into sin)."""
                t1 = rpool.tile([128, CH], F32, tag="r1", name="t1")
                t2 = rpool.tile([128, CH], F32, tag="r2", name="t2")
                nc.vector.tensor_mul(t1, ps, cos_ap)
                nc.vector.tensor_mul(t2[0:64, :], ps[64:128, :], sin_ap[0:64, :])
                nc.vector.tensor_mul(t2[64:128, :], ps[0:64, :], sin_ap[64:128, :])
                nc.vector.tensor_add(out_ap, t1, t2)

            def emit_chunk(c):
                t0 = c * CH
                hs_sb = hpool.tile([128, KT, CH], BF16, tag="hs", name="hs_sb")
                for g in range(4):
                    nc.sync.dma_start(hs_sb[:, g * 8:(g + 1) * 8, :],
                                      hsT_r[:, g * 8:(g + 1) * 8, t0:t0 + CH])
                for m in range(HL):
                    ps = wpool.tile([128, CH], F32, tag="work", name="ps_q")
                    for kt in range(KT):
                        nc.tensor.matmul(ps, wq_h[m][:, kt, :], hs_sb[:, kt, :],
                                         start=(kt == 0), stop=(kt == KT - 1))
                    if c == 0 and m == 0:
                        late_consts()
                    rope(ps, qt_rot[:, m, t0:t0 + CH],
                         cq_sb[:, t0:t0 + CH], sq_sb[:, t0:t0 + CH])
                ps = wpool.tile([128, CH], F32, tag="work", name="ps_k")
                for kt in range(KT):
                    nc.tensor.matmul(ps, wk_sb[:, kt, :], hs_sb[:, kt, :],
                                     start=(kt == 0), stop=(kt == KT - 1))
                rope(ps, kt_rot[:, t0:t0 + CH],
                     ck_sb[:, t0:t0 + CH], sk_sb[:, t0:t0 + CH])
                for vi in range(CH // 128):
                    tt = t0 // 128 + vi
                    ps = wpool.tile([128, HD], F32, tag="work", name="ps_v")
                    for kt in range(KT):
                        nc.tensor.matmul(ps, hs_sb[:, kt, vi * 128:(vi + 1) * 128],
                                         wv_sb[:, kt, :],
                                         start=(kt == 0), stop=(kt == KT - 1))
                    nc.scalar.copy(v_sb[:, tt, :], ps)

            # --- attention group machinery (transposed-scores scheme) ---
            pending = [None]

            def epilogue(st):
                rs, ot, h, q0 = st
                recip = rcpool.tile([1, QB], F32, tag="recip", name="recip")
                nc.vector.reciprocal(recip, rs)
                bcs = bcsbpool.tile([128, QB], F32, tag="bcs", name="bcs")
                nc.gpsimd.partition_broadcast(bcs, recip)
                nc.vector.tensor_mul(ao[:, h, q0:q0 + QB], ot, bcs)

            def emit_group(b, h, qb):
                q0 = b * S + qb * QB
                n_kt = (qb + 1) * (QB // 128)
                rs = rspool.tile([1, QB], F32, tag="rs", name="rs")
                ot = otpool.tile([128, QB], F32, tag="ot", name="ot")
                ets = [None] * n_kt

                def emit_sc(kt):
                    c0 = max(0, kt - qb * (QB // 128)) * 128
                    sc = wpool.tile([128, QB], F32, tag="work", name="sc")
                    nc.tensor.matmul(
                        sc[:, c0:],
                        kt_rot[:, b * S + kt * 128:b * S + (kt + 1) * 128],
                        qt_rot[:, h, q0 + c0:q0 + QB],
                        start=True, stop=True)
                    jd = kt - qb * (QB // 128)
                    if 0 <= jd < QB // 128:
                        nc.vector.tensor_add(sc[:, jd * 128:(jd + 1) * 128],
                                             sc[:, jd * 128:(jd + 1) * 128],
                                             maskT_sb)
                    et = epool.tile([128, QB], BF16, tag="et", name="et")
                    nc.scalar.activation(et[:, c0:], sc[:, c0:], Exp,
                                         bias=0.0, scale=1.0)
                    ets[kt] = (et, c0)

                emit_sc(0)
                if n_kt > 1:
                    emit_sc(1)
                for kt in range(n_kt):
                    if kt + 2 < n_kt:
                        emit_sc(kt + 2)
                    et, c0 = ets[kt]
                    nc.tensor.matmul(rs[:, c0:], ones_k, et[:, c0:],
                                     start=(kt == 0), stop=(kt == n_kt - 1))
                    nc.tensor.matmul(ot[:, c0:], v_sb[:, b * (S // 128) + kt, :],
                                     et[:, c0:], start=(kt == 0),
                                     stop=(kt == n_kt - 1))
                    ets[kt] = None
                    if kt == 0 and pending[0] is not None:
                        epilogue(pending[0])
                        pending[0] = None
                pending[0] = (rs, ot, h, q0)

            # --- interleaved emission: each chunk unblocks a set of groups ---
            # chunk c covers tokens [c*512, (c+1)*512) = batch c//2, q-block c%2
            wo_sb = None
            for c in range(NCHUNK):
                emit_chunk(c)
                b, qb = c // 2, c % 2
                for h in range(HL):
                    emit_group(b, h, qb)
                if c == NCHUNK - 1:
                    # wo reuses an hs slot (same size); DMA overlaps the
                    # final attention groups
                    wo_sb = hpool.tile([128, HL, DIM], BF16, tag="hs",
                                       name="wo_sb")
                    nc.sync.dma_start(
                        wo_sb, wo.ap().rearrange("(kh p) n -> p kh n", p=128))
            if pending[0] is not None:
                epilogue(pending[0])
                pending[0] = None

            # ---- output projection (row-parallel Wo) ----
            for tt in range(TT):
                for ni, n0 in enumerate(range(0, DIM, 512)):
                    ps = wpool.tile([128, 512], F32, tag="work", name="ps_o")
                    for kh in range(HL):
                        nc.tensor.matmul(ps, ao[:, kh, tt * 128:(tt + 1) * 128],
                                         wo_sb[:, kh, n0:n0 + 512],
                                         start=(kh == 0), stop=(kh == HL - 1))
                    osb = xsbpool.tile([128, 512], BF16, tag="osb", name="osb")
                    if (tt * 8 + ni) % 2 == 0:
                        nc.scalar.copy(osb, ps)
                    else:
                        nc.vector.tensor_copy(osb, ps)
                    nc.sync.dma_start(out.ap()[tt * 128:(tt + 1) * 128,
                                               n0:n0 + 512], osb)
    nc.compile()
    return nc


def _get_nc():
    if "nc" not in _CACHE:
        _CACHE["nc"] = _build()
    return _CACHE["nc"]


def _prep_inputs(inputs) -> list[dict]:
    bf16 = ml_dtypes.bfloat16
    hs = np.asarray(inputs["hidden_states"], dtype=np.float32).reshape(T, DIM)
    hsT = np.ascontiguousarray(hs.T).astype(bf16)

    fc = np.asarray(inputs["freqs_cos"], dtype=np.float32).reshape(T, HD // 2).T
    fs = np.asarray(inputs["freqs_sin"], dtype=np.float32).reshape(T, HD // 2).T
    cos2 = np.concatenate([fc, fc], axis=0)            # [128, T]
    sin2 = np.concatenate([-fs, fs], axis=0)           # signed half-rotation
    cos_qv = np.ascontiguousarray(cos2 * SCALE).astype(bf16)
    sin_qv = np.ascontiguousarray(sin2 * SCALE).astype(bf16)
    cos_kv = np.ascontiguousarray(cos2).astype(bf16)
    sin_kv = np.ascontiguousarray(sin2).astype(bf16)

    maskT = np.ascontiguousarray(
        np.asarray(inputs["attention_mask"], dtype=np.float32)[0, 0, :128, :128].T)

    perm = np.concatenate([np.arange(0, HD, 2), np.arange(1, HD, 2)])
    Wq = np.asarray(inputs["Wq"], dtype=np.float32)
    Wk = np.asarray(inputs["Wk"], dtype=np.float32)
    Wv = np.asarray(inputs["Wv"], dtype=np.float32)
    Wo = np.asarray(inputs["Wo"], dtype=np.float32)

    in_maps = []
    for c in range(N_CORES):
        wq_c = np.concatenate(
            [Wq[:, (c * HL + h) * HD:(c * HL + h + 1) * HD][:, perm]
             for h in range(HL)], axis=1)
        wk_c = Wk[:, c * HD:(c + 1) * HD][:, perm]
        wv_c = Wv[:, c * HD:(c + 1) * HD]
        wo_c = Wo[c * HL * HD:(c + 1) * HL * HD, :]
        in_maps.append({
            "hsT": hsT,
            "wq": np.ascontiguousarray(wq_c).astype(bf16),
            "wk": np.ascontiguousarray(wk_c).astype(bf16),
            "wv": np.ascontiguousarray(wv_c).astype(bf16),
            "wo": np.ascontiguousarray(wo_c).astype(bf16),
            "cos_q": cos_qv, "sin_q": sin_qv,
            "cos_k": cos_kv, "sin_k": sin_kv,
            "maskT": maskT,
        })
    return in_maps


def kernel(**inputs) -> np.ndarray:
    nc = _get_nc()
    in_maps = _prep_inputs(inputs)
    res = bass_utils.run_bass_kernel_spmd(nc, in_maps,
                                          core_ids=list(range(N_CORES)))
    acc = np.zeros((T, DIM), dtype=np.float32)
    for c in range(N_CORES):
        acc += np.asarray(res.results[c]["out"], dtype=np.float32)
    return acc.reshape(B, S, DIM)


# revision 27
# speedup vs baseline: 1.0010x; 1.0010x over previous
"""Trainium2 Bass kernel for MllamaTextSdpaAttention (GQA + RoPE + causal SDPA).

Strategy: tensor-parallel over heads across 8 NeuronCores. Core c owns
q-heads [4c, 4c+4) and kv-head c (kv groups intact). Each core computes
hidden @ Wq/Wk/Wv slices, RoPE, causal attention for its heads, and its
row-slice of the Wo matmul, yielding a partial [T, DIM] output (bf16).
The host sums the 8 partials in f32.

Layout tricks:
- hidden_states is fed transposed ([DIM, T], bf16) so every projection
  matmul has the contraction dim (features) on partitions.
- Q/K projections produce Q^T/K^T directly (head_dim=128 on partitions).
- The RoPE even/odd pairing is de-interleaved by permuting Wq/Wk columns
  on the host, turning RoPE into a half-rotation: the partner element sits
  64 partitions away, reachable with plain partition-offset slices. The
  d-permutation cancels in q.k^T. The 1/sqrt(d) scale is folded into Q's
  cos/sin tables.
- Scores are computed TRANSPOSED: scT[k, q] = K_rot^T(tile).T @ Q_rot^T.
  exp(scT) is then directly the moving operand for the P@V matmul
  (out^T[d,q] = V[k,d].T @ expT[k,q]) -- no P transposes or PSUM->SBUF
  P copies. Softmax denominators come from a ones-vector matmul on the
  PE (sum over k = partition dim), and the 1/sum normalization is applied
  to the small out^T tile (via a PE-broadcast of the reciprocal row),
  not to P. No max-subtraction: scores are bounded (|s| <= ~20) so f32
  exp is safe, and masked entries use the additive -1e9 mask -> exp = 0.
- Causality at 128-block granularity: k-blocks strictly above the
  diagonal are never computed or read; diagonal blocks get the transposed
  additive mask from the actual attention_mask input.
- The 1/rowsum reciprocal row is broadcast across partitions on the idle
  GpSimd engine (partition_broadcast), and each group's normalization
  epilogue is deferred into the next group (software pipelining) so the
  PE never waits on the DVE reciprocal.
- Emission interleaves projection chunks with the attention groups they
  unblock (chunk0 -> b0/qb0 groups -> chunk1 -> b0/qb1 groups -> ...) and
  all [128,512]-f32 PSUM scratch (projection accumulators, score tiles,
  output accumulators) shares one 5-slot pool (+2 ot +1 rs = 8 banks)
  so the whole kernel fits PSUM without phase barriers.
- TimelineSim (instruction cost model): ~353 us/core; PE busy ~326 us
  (92% occupancy), which is the bf16 matmul-column floor for this
  decomposition.
"""

import numpy as np
import ml_dtypes

import concourse.bacc as bacc
import concourse.bass as bass
import concourse.mybir as mybir
from concourse.tile import TileContext
from concourse import bass_utils

BF16 = mybir.dt.bfloat16
F32 = mybir.dt.float32

B, S, DIM = 2, 1024, 4096
T = B * S                     # 2048 tokens, batch-major
N_HEADS, N_KV = 32, 8
HD = 128                      # head dim == partition count
N_CORES = 8
HL = N_HEADS // N_CORES       # 4 local q-heads per core
KT = DIM // 128               # 32 feature tiles
CH = 512                      # projection token-chunk
NCHUNK = T // CH
QB = 512                      # attention q-block width
TT = T // 128                 # 16 token tiles global
SCALE = 1.0 / float(np.sqrt(HD))

_CACHE: dict = {}


def _build():
    nc = bacc.Bacc("TRN2", target_bir_lowering=False, debug=False,
                   enable_asserts=False)

    hsT = nc.dram_tensor("hsT", [DIM, T], BF16, kind="ExternalInput")
    wq = nc.dram_tensor("wq", [DIM, HL * HD], BF16, kind="ExternalInput")
    wk = nc.dram_tensor("wk", [DIM, HD], BF16, kind="ExternalInput")
    wv = nc.dram_tensor("wv", [DIM, HD], BF16, kind="ExternalInput")
    wo = nc.dram_tensor("wo", [HL * HD, DIM], BF16, kind="ExternalInput")
    cos_q = nc.dram_tensor("cos_q", [HD, T], BF16, kind="ExternalInput")
    sin_q = nc.dram_tensor("sin_q", [HD, T], BF16, kind="ExternalInput")
    cos_k = nc.dram_tensor("cos_k", [HD, T], BF16, kind="ExternalInput")
    sin_k = nc.dram_tensor("sin_k", [HD, T], BF16, kind="ExternalInput")
    maskT = nc.dram_tensor("maskT", [128, 128], F32, kind="ExternalInput")
    out = nc.dram_tensor("out", [T, DIM], BF16, kind="ExternalOutput")

    Exp = mybir.ActivationFunctionType.Exp

    with TileContext(nc) as tc:
        with tc.tile_pool(name="consts", bufs=1) as cpool, \
             tc.tile_pool(name="hs", bufs=2) as hpool, \
             tc.tile_pool(name="rope_tmp", bufs=2) as rpool, \
             tc.tile_pool(name="work_ps", bufs=5, space=bass.MemorySpace.PSUM) as wpool, \
             tc.tile_pool(name="ot_ps", bufs=2, space=bass.MemorySpace.PSUM) as otpool, \
             tc.tile_pool(name="rs_ps", bufs=1, space=bass.MemorySpace.PSUM) as rspool, \
             tc.tile_pool(name="et", bufs=6) as epool, \
             tc.tile_pool(name="bc_sb", bufs=2) as bcsbpool, \
             tc.tile_pool(name="recip", bufs=4) as rcpool, \
             tc.tile_pool(name="out_sb", bufs=4) as xsbpool:

            wq_h = [cpool.tile([128, KT, HD], BF16, tag=f"wq{m}", name=f"wq{m}")
                    for m in range(HL)]
            wk_sb = cpool.tile([128, KT, HD], BF16, tag="wk")
            wv_sb = cpool.tile([128, KT, HD], BF16, tag="wv")
            cq_sb = cpool.tile([128, T], BF16, tag="cq")
            sq_sb = cpool.tile([128, T], BF16, tag="sq")
            ck_sb = cpool.tile([128, T], BF16, tag="ck")
            sk_sb = cpool.tile([128, T], BF16, tag="sk")
            maskT_sb = cpool.tile([128, 128], F32, tag="maskT")
            ones_k = cpool.tile([128, 1], BF16, tag="ones_k")
            qt_rot = cpool.tile([128, HL, T], BF16, tag="qt")
            kt_rot = cpool.tile([128, T], BF16, tag="kt")
            v_sb = cpool.tile([128, TT, HD], BF16, tag="v")
            ao = cpool.tile([128, HL, T], BF16, tag="ao")

            wq_r = wq.ap().rearrange("(kt p) n -> p kt n", p=128)
            hsT_r = hsT.ap().rearrange("(kt p) t -> p kt t", p=128)

            # startup-critical DMA first: the k-tiles the first matmuls touch
            nc.sync.dma_start(wq_h[0][:, 0:8, :], wq_r[:, 0:8, 0:HD])
            nc.sync.dma_start(wq_h[0][:, 8:KT, :], wq_r[:, 8:KT, 0:HD])

            def late_consts():
                nc.sync.dma_start(wq_h[1], wq_r[:, :, HD:2 * HD])
                nc.sync.dma_start(cq_sb, cos_q.ap())
                nc.sync.dma_start(sq_sb, sin_q.ap())
                for m in range(2, HL):
                    nc.sync.dma_start(wq_h[m], wq_r[:, :, m * HD:(m + 1) * HD])
                nc.sync.dma_start(wk_sb, wk.ap().rearrange("(kt p) n -> p kt n", p=128))
                nc.sync.dma_start(ck_sb, cos_k.ap())
                nc.sync.dma_start(sk_sb, sin_k.ap())
                nc.sync.dma_start(wv_sb, wv.ap().rearrange("(kt p) n -> p kt n", p=128))
                nc.sync.dma_start(maskT_sb, maskT.ap())
                nc.vector.memset(ones_k, 1.0)

            def rope(ps, out_ap, cos_ap, sin_ap):
                """out = ps*cos + halfswap(ps)*sin  (signs baked into sin)."""
                t1 = rpool.tile([128, CH], F32, tag="r1", name="t1")
                t2 = rpool.tile([128, CH], F32, tag="r2", name="t2")
                nc.vector.tensor_mul(t1, ps, cos_ap)
                nc.vector.tensor_mul(t2[0:64, :], ps[64:128, :], sin_ap[0:64, :])
                nc.vector.tensor_mul(t2[64:128, :], ps[0:64, :], sin_ap[64:128, :])
                nc.vector.tensor_add(out_ap, t1, t2)

            def emit_chunk(c):
                t0 = c * CH
                hs_sb = hpool.tile([128, KT, CH], BF16, tag="hs", name="hs_sb")
                for g in range(4):
                    nc.sync.dma_start(hs_sb[:, g * 8:(g + 1) * 8, :],
                                      hsT_r[:, g * 8:(g + 1) * 8, t0:t0 + CH])
                for m in range(HL):
                    ps = wpool.tile([128, CH], F32, tag="work", name="ps_q")
                    for kt in range(KT):
                        nc.tensor.matmul(ps, wq_h[m][:, kt, :], hs_sb[:, kt, :],
                                         start=(kt == 0), stop=(kt == KT - 1))
                    if c == 0 and m == 0:
                        late_consts()
                    rope(ps, qt_rot[:, m, t0:t0 + CH],
                         cq_sb[:, t0:t0 + CH], sq_sb[:, t0:t0 + CH])
                ps = wpool.tile([128, CH], F32, tag="work", name="ps_k")
                for kt in range(KT):
                    nc.tensor.matmul(ps, wk_sb[:, kt, :], hs_sb[:, kt, :],
                                     start=(kt == 0), stop=(kt == KT - 1))
                rope(ps, kt_rot[:, t0:t0 + CH],
                     ck_sb[:, t0:t0 + CH], sk_sb[:, t0:t0 + CH])
                for vi in range(CH // 128):
                    tt = t0 // 128 + vi
                    ps = wpool.tile([128, HD], F32, tag="work", name="ps_v")
                    for kt in range(KT):
                        nc.tensor.matmul(ps, hs_sb[:, kt, vi * 128:(vi + 1) * 128],
                                         wv_sb[:, kt, :],
                                         start=(kt == 0), stop=(kt == KT - 1))
                    nc.scalar.copy(v_sb[:, tt, :], ps)

            # --- attention group machinery (transposed-scores scheme) ---
            pending = [None]

            def epilogue(st):
                rs, ot, h, q0 = st
                recip = rcpool.tile([1, QB], F32, tag="recip", name="recip")
                nc.vector.reciprocal(recip, rs)
                bcs = bcsbpool.tile([128, QB], F32, tag="bcs", name="bcs")
                nc.gpsimd.partition_broadcast(bcs, recip)
                nc.vector.tensor_mul(ao[:, h, q0:q0 + QB], ot, bcs)

            def emit_group(b, h, qb):
                q0 = b * S + qb * QB
                n_kt = (qb + 1) * (QB // 128)
                rs = rspool.tile([1, QB], F32, tag="rs", name="rs")
                ot = otpool.tile([128, QB], F32, tag="ot", name="ot")
                ets = [None] * n_kt

                def emit_sc(kt):
                    c0 = max(0, kt - qb * (QB // 128)) * 128
                    sc = wpool.tile([128, QB], F32, tag="work", name="sc")
                    nc.tensor.matmul(
                        sc[:, c0:],
                        kt_rot[:, b * S + kt * 128:b * S + (kt + 1) * 128],
                        qt_rot[:, h, q0 + c0:q0 + QB],
                        start=True, stop=True)
                    jd = kt - qb * (QB // 128)
                    if 0 <= jd < QB // 128:
                        nc.vector.tensor_add(sc[:, jd * 128:(jd + 1) * 128],
                                             sc[:, jd * 128:(jd + 1) * 128],
                                             maskT_sb)
                    et = epool.tile([128, QB], BF16, tag="et", name="et")
                    nc.scalar.activation(et[:, c0:], sc[:, c0:], Exp,
                                         bias=0.0, scale=1.0)
                    ets[kt] = (et, c0)

                for w in range(min(3, n_kt)):
                    emit_sc(w)
                for kt in range(n_kt):
                    if kt + 3 < n_kt:
                        emit_sc(kt + 3)
                    et, c0 = ets[kt]
                    nc.tensor.matmul(rs[:, c0:], ones_k, et[:, c0:],
                                     start=(kt == 0), stop=(kt == n_kt - 1))
                    nc.tensor.matmul(ot[:, c0:], v_sb[:, b * (S // 128) + kt, :],
                                     et[:, c0:], start=(kt == 0),
                                     stop=(kt == n_kt - 1))
                    ets[kt] = None
                    if kt == 0 and pending[0] is not None:
                        epilogue(pending[0])
                        pending[0] = None
                pending[0] = (rs, ot, h, q0)

            # --- interleaved emission: each chunk unblocks a set of groups ---
            # chunk c covers tokens [c*512, (c+1)*512) = batch c//2, q-block c%2
            wo_sb = None
            for c in range(NCHUNK):
                emit_chunk(c)
                b, qb = c // 2, c % 2
                for h in range(HL):
                    emit_group(b, h, qb)
                if c == NCHUNK - 1:
                    # wo reuses an hs slot (same size); DMA overlaps the
                    # final attention groups
                    wo_sb = hpool.tile([128, HL, DIM], BF16, tag="hs",
                                       name="wo_sb")
                    nc.sync.dma_start(
                        wo_sb, wo.ap().rearrange("(kh p) n -> p kh n", p=128))
            if pending[0] is not None:
                epilogue(pending[0])
                pending[0] = None

            # ---- output projection (row-parallel Wo) ----
            for tt in range(TT):
                for ni, n0 in enumerate(range(0, DIM, 512)):
                    ps = wpool.tile([128, 512], F32, tag="work", name="ps_o")
                    for kh in range(HL):
                        nc.tensor.matmul(ps, ao[:, kh, tt * 128:(tt + 1) * 128],
                                         wo_sb[:, kh, n0:n0 + 512],
                                         start=(kh == 0), stop=(kh == HL - 1))
                    osb = xsbpool.tile([128, 512], BF16, tag="osb", name="osb")
                    if (tt * 8 + ni) % 2 == 0:
                        nc.scalar.copy(osb, ps)
                    else:
                        nc.vector.tensor_copy(osb, ps)
                    nc.sync.dma_start(out.ap()[tt * 128:(tt + 1) * 128,
                                               n0:n0 + 512], osb)
    nc.compile()
    return nc


def _get_nc():
    if "nc" not in _CACHE:
        _CACHE["nc"] = _build()
    return _CACHE["nc"]


def _prep_inputs(inputs) -> list[dict]:
    bf16 = ml_dtypes.bfloat16
    hs = np.asarray(inputs["hidden_states"], dtype=np.float32).reshape(T, DIM)
    hsT = np.ascontiguousarray(hs.T).astype(bf16)

    fc = np.asarray(inputs["freqs_cos"], dtype=np.float32).reshape(T, HD // 2).T
    fs = np.asarray(inputs["freqs_sin"], dtype=np.float32).reshape(T, HD // 2).T
    cos2 = np.concatenate([fc, fc], axis=0)            # [128, T]
    sin2 = np.concatenate([-fs, fs], axis=0)           # signed half-rotation
    cos_qv = np.ascontiguousarray(cos2 * SCALE).astype(bf16)
    sin_qv = np.ascontiguousarray(sin2 * SCALE).astype(bf16)
    cos_kv = np.ascontiguousarray(cos2).astype(bf16)
    sin_kv = np.ascontiguousarray(sin2).astype(bf16)

    maskT = np.ascontiguousarray(
        np.asarray(inputs["attention_mask"], dtype=np.float32)[0, 0, :128, :128].T)

    perm = np.concatenate([np.arange(0, HD, 2), np.arange(1, HD, 2)])
    Wq = np.asarray(inputs["Wq"], dtype=np.float32)
    Wk = np.asarray(inputs["Wk"], dtype=np.float32)
    Wv = np.asarray(inputs["Wv"], dtype=np.float32)
    Wo = np.asarray(inputs["Wo"], dtype=np.float32)

    in_maps = []
    for c in range(N_CORES):
        wq_c = np.concatenate(
            [Wq[:, (c * HL + h) * HD:(c * HL + h + 1) * HD][:, perm]
             for h in range(HL)], axis=1)
        wk_c = Wk[:, c * HD:(c + 1) * HD][:, perm]
        wv_c = Wv[:, c * HD:(c + 1) * HD]
        wo_c = Wo[c * HL * HD:(c + 1) * HL * HD, :]
        in_maps.append({
            "hsT": hsT,
            "wq": np.ascontiguousarray(wq_c).astype(bf16),
            "wk": np.ascontiguousarray(wk_c).astype(bf16),
            "wv": np.ascontiguousarray(wv_c).astype(bf16),
            "wo": np.ascontiguousarray(wo_c).astype(bf16),
            "cos_q": cos_qv, "sin_q": sin_qv,
            "cos_k": cos_kv, "sin_k": sin_kv,
            "maskT": maskT,
        })
    return in_maps


def kernel(**inputs) -> np.ndarray:
    nc = _get_nc()
    in_maps = _prep_inputs(inputs)
    res = bass_utils.run_bass_kernel_spmd(nc, in_maps,
                                          core_ids=list(range(N_CORES)))
    acc = np.zeros((T, DIM), dtype=np.float32)
    for c in range(N_CORES):
        acc += np.asarray(res.results[c]["out"], dtype=np.float32)
    return acc.reshape(B, S, DIM)


# revision 30
# speedup vs baseline: 1.0078x; 1.0068x over previous
"""Trainium2 Bass kernel for MllamaTextSdpaAttention (GQA + RoPE + causal SDPA).

Strategy: tensor-parallel over heads across 8 NeuronCores. Core c owns
q-heads [4c, 4c+4) and kv-head c (kv groups intact). Each core computes
hidden @ Wq/Wk/Wv slices, RoPE, causal attention for its heads, and its
row-slice of the Wo matmul, yielding a partial [T, DIM] output (bf16).
The host sums the 8 partials in f32.

Layout tricks:
- hidden_states is fed transposed ([DIM, T], bf16) so every projection
  matmul has the contraction dim (features) on partitions.
- Q/K projections produce Q^T/K^T directly (head_dim=128 on partitions).
- The RoPE even/odd pairing is de-interleaved by permuting Wq/Wk columns
  on the host, turning RoPE into a half-rotation: the partner element sits
  64 partitions away, reachable with plain partition-offset slices. The
  d-permutation cancels in q.k^T. The 1/sqrt(d) scale is folded into Q's
  cos/sin tables.
- Scores are computed TRANSPOSED: scT[k, q] = K_rot^T(tile).T @ Q_rot^T.
  exp(scT) is then directly the moving operand for the P@V matmul
  (out^T[d,q] = V[k,d].T @ expT[k,q]) -- no P transposes or PSUM->SBUF
  P copies. Softmax denominators come from a ones-vector matmul on the
  PE (sum over k = partition dim), and the 1/sum normalization is applied
  to the small out^T tile (via a PE-broadcast of the reciprocal row),
  not to P. No max-subtraction: scores are bounded (|s| <= ~20) so f32
  exp is safe, and masked entries use the additive -1e9 mask -> exp = 0.
- Causality at 128-block granularity: k-blocks strictly above the
  diagonal are never computed or read; diagonal blocks get the transposed
  additive mask from the actual attention_mask input.
- The 1/rowsum reciprocal row is broadcast across partitions on the idle
  GpSimd engine (partition_broadcast), and each group's normalization
  epilogue is deferred into the next group (software pipelining) so the
  PE never waits on the DVE reciprocal.
- Emission interleaves projection chunks with the attention groups they
  unblock (chunk0 -> b0/qb0 groups -> chunk1 -> b0/qb1 groups -> ...) and
  all [128,512]-f32 PSUM scratch (projection accumulators, score tiles,
  output accumulators) shares one 5-slot pool (+2 ot +1 rs = 8 banks)
  so the whole kernel fits PSUM without phase barriers.
- TimelineSim (instruction cost model): ~353 us/core; PE busy ~326 us
  (92% occupancy), which is the bf16 matmul-column floor for this
  decomposition.
"""

import numpy as np
import ml_dtypes

import concourse.bacc as bacc
import concourse.bass as bass
import concourse.mybir as mybir
from concourse.tile import TileContext
from concourse import bass_utils

BF16 = mybir.dt.bfloat16
F32 = mybir.dt.float32

B, S, DIM = 2, 1024, 4096
T = B * S                     # 2048 tokens, batch-major
N_HEADS, N_KV = 32, 8
HD = 128                      # head dim == partition count
N_CORES = 8
HL = N_HEADS // N_CORES       # 4 local q-heads per core
KT = DIM // 128               # 32 feature tiles
CH = 512                      # projection token-chunk
NCHUNK = T // CH
QB = 512                      # attention q-block width
TT = T // 128                 # 16 token tiles global
SCALE = 1.0 / float(np.sqrt(HD))

_CACHE: dict = {}


def _build():
    nc = bacc.Bacc("TRN2", target_bir_lowering=False, debug=False,
                   enable_asserts=False)

    hsT = nc.dram_tensor("hsT", [DIM, T], BF16, kind="ExternalInput")
    wq = nc.dram_tensor("wq", [DIM, HL * HD], BF16, kind="ExternalInput")
    wk = nc.dram_tensor("wk", [DIM, HD], BF16, kind="ExternalInput")
    wv = nc.dram_tensor("wv", [DIM, HD], BF16, kind="ExternalInput")
    wo = nc.dram_tensor("wo", [HL * HD, DIM], BF16, kind="ExternalInput")
    cos_q = nc.dram_tensor("cos_q", [HD, T], BF16, kind="ExternalInput")
    sin_q = nc.dram_tensor("sin_q", [HD, T], BF16, kind="ExternalInput")
    cos_k = nc.dram_tensor("cos_k", [HD, T], BF16, kind="ExternalInput")
    sin_k = nc.dram_tensor("sin_k", [HD, T], BF16, kind="ExternalInput")
    maskT = nc.dram_tensor("maskT", [128, 128], F32, kind="ExternalInput")
    out = nc.dram_tensor("out", [T, DIM], BF16, kind="ExternalOutput")

    Exp = mybir.ActivationFunctionType.Exp

    with TileContext(nc) as tc:
        with tc.tile_pool(name="consts", bufs=1) as cpool, \
             tc.tile_pool(name="hs", bufs=2) as hpool, \
             tc.tile_pool(name="rope_tmp", bufs=2) as rpool, \
             tc.tile_pool(name="work_ps", bufs=5, space=bass.MemorySpace.PSUM) as wpool, \
             tc.tile_pool(name="ot_ps", bufs=2, space=bass.MemorySpace.PSUM) as otpool, \
             tc.tile_pool(name="rs_ps", bufs=1, space=bass.MemorySpace.PSUM) as rspool, \
             tc.tile_pool(name="et", bufs=6) as epool, \
             tc.tile_pool(name="bc_sb", bufs=2) as bcsbpool, \
             tc.tile_pool(name="recip", bufs=4) as rcpool, \
             tc.tile_pool(name="out_sb", bufs=6) as xsbpool:

            wq_h = [cpool.tile([128, KT, HD], BF16, tag=f"wq{m}", name=f"wq{m}")
                    for m in range(HL)]
            wk_sb = cpool.tile([128, KT, HD], BF16, tag="wk")
            wv_sb = cpool.tile([128, KT, HD], BF16, tag="wv")
            cq_sb = cpool.tile([128, T], BF16, tag="cq")
            sq_sb = cpool.tile([128, T], BF16, tag="sq")
            ck_sb = cpool.tile([128, T], BF16, tag="ck")
            sk_sb = cpool.tile([128, T], BF16, tag="sk")
            maskT_sb = cpool.tile([128, 128], F32, tag="maskT")
            ones_k = cpool.tile([128, 1], BF16, tag="ones_k")
            qt_rot = cpool.tile([128, HL, T], BF16, tag="qt")
            kt_rot = cpool.tile([128, T], BF16, tag="kt")
            v_sb = cpool.tile([128, TT, HD], BF16, tag="v")
            ao = cpool.tile([128, HL, T], BF16, tag="ao")

            wq_r = wq.ap().rearrange("(kt p) n -> p kt n", p=128)
            hsT_r = hsT.ap().rearrange("(kt p) t -> p kt t", p=128)

            # startup-critical DMA first: the k-tiles the first matmuls touch
            nc.sync.dma_start(wq_h[0][:, 0:8, :], wq_r[:, 0:8, 0:HD])
            nc.sync.dma_start(wq_h[0][:, 8:KT, :], wq_r[:, 8:KT, 0:HD])

            def late_consts():
                nc.sync.dma_start(wq_h[1], wq_r[:, :, HD:2 * HD])
                nc.sync.dma_start(cq_sb, cos_q.ap())
                nc.sync.dma_start(sq_sb, sin_q.ap())
                for m in range(2, HL):
                    nc.sync.dma_start(wq_h[m], wq_r[:, :, m * HD:(m + 1) * HD])
                nc.sync.dma_start(wk_sb, wk.ap().rearrange("(kt p) n -> p kt n", p=128))
                nc.sync.dma_start(ck_sb, cos_k.ap())
                nc.sync.dma_start(sk_sb, sin_k.ap())
                nc.sync.dma_start(wv_sb, wv.ap().rearrange("(kt p) n -> p kt n", p=128))
                nc.sync.dma_start(maskT_sb, maskT.ap())
                nc.vector.memset(ones_k, 1.0)

            def rope(ps, out_ap, cos_ap, sin_ap):
                """out = ps*cos + halfswap(ps)*sin  (signs baked into sin)."""
                t1 = rpool.tile([128, CH], F32, tag="r1", name="t1")
                t2 = rpool.tile([128, CH], F32, tag="r2", name="t2")
                nc.vector.tensor_mul(t1, ps, cos_ap)
                nc.vector.tensor_mul(t2[0:64, :], ps[64:128, :], sin_ap[0:64, :])
                nc.vector.tensor_mul(t2[64:128, :], ps[0:64, :], sin_ap[64:128, :])
                nc.vector.tensor_add(out_ap, t1, t2)

            def emit_chunk(c):
                t0 = c * CH
                hs_sb = hpool.tile([128, KT, CH], BF16, tag="hs", name="hs_sb")
                for g in range(4):
                    nc.sync.dma_start(hs_sb[:, g * 8:(g + 1) * 8, :],
                                      hsT_r[:, g * 8:(g + 1) * 8, t0:t0 + CH])
                for m in range(HL):
                    ps = wpool.tile([128, CH], F32, tag="work", name="ps_q")
                    for kt in range(KT):
                        nc.tensor.matmul(ps, wq_h[m][:, kt, :], hs_sb[:, kt, :],
                                         start=(kt == 0), stop=(kt == KT - 1))
                    if c == 0 and m == 0:
                        late_consts()
                    rope(ps, qt_rot[:, m, t0:t0 + CH],
                         cq_sb[:, t0:t0 + CH], sq_sb[:, t0:t0 + CH])
                ps = wpool.tile([128, CH], F32, tag="work", name="ps_k")
                for kt in range(KT):
                    nc.tensor.matmul(ps, wk_sb[:, kt, :], hs_sb[:, kt, :],
                                     start=(kt == 0), stop=(kt == KT - 1))
                rope(ps, kt_rot[:, t0:t0 + CH],
                     ck_sb[:, t0:t0 + CH], sk_sb[:, t0:t0 + CH])
                for vi in range(CH // 128):
                    tt = t0 // 128 + vi
                    ps = wpool.tile([128, HD], F32, tag="work", name="ps_v")
                    for kt in range(KT):
                        nc.tensor.matmul(ps, hs_sb[:, kt, vi * 128:(vi + 1) * 128],
                                         wv_sb[:, kt, :],
                                         start=(kt == 0), stop=(kt == KT - 1))
                    nc.scalar.copy(v_sb[:, tt, :], ps)

            # --- attention group machinery (transposed-scores scheme) ---
            pending = [None]

            def epilogue(st):
                rs, ot, h, q0 = st
                recip = rcpool.tile([1, QB], F32, tag="recip", name="recip")
                nc.vector.reciprocal(recip, rs)
                bcs = bcsbpool.tile([128, QB], F32, tag="bcs", name="bcs")
                nc.gpsimd.partition_broadcast(bcs, recip)
                nc.vector.tensor_mul(ao[:, h, q0:q0 + QB], ot, bcs)

            def emit_group(b, h, qb):
                q0 = b * S + qb * QB
                n_kt = (qb + 1) * (QB // 128)
                rs = rspool.tile([1, QB], F32, tag="rs", name="rs")
                ot = otpool.tile([128, QB], F32, tag="ot", name="ot")
                ets = [None] * n_kt

                def emit_sc(kt):
                    c0 = max(0, kt - qb * (QB // 128)) * 128
                    sc = wpool.tile([128, QB], F32, tag="work", name="sc")
                    nc.tensor.matmul(
                        sc[:, c0:],
                        kt_rot[:, b * S + kt * 128:b * S + (kt + 1) * 128],
                        qt_rot[:, h, q0 + c0:q0 + QB],
                        start=True, stop=True)
                    jd = kt - qb * (QB // 128)
                    if 0 <= jd < QB // 128:
                        nc.vector.tensor_add(sc[:, jd * 128:(jd + 1) * 128],
                                             sc[:, jd * 128:(jd + 1) * 128],
                                             maskT_sb)
                    et = epool.tile([128, QB], BF16, tag="et", name="et")
                    nc.scalar.activation(et[:, c0:], sc[:, c0:], Exp,
                                         bias=0.0, scale=1.0)
                    ets[kt] = (et, c0)

                for w in range(min(4, n_kt)):
                    emit_sc(w)
                for kt in range(n_kt):
                    if kt + 4 < n_kt:
                        emit_sc(kt + 4)
                    et, c0 = ets[kt]
                    nc.tensor.matmul(rs[:, c0:], ones_k, et[:, c0:],
                                     start=(kt == 0), stop=(kt == n_kt - 1))
                    nc.tensor.matmul(ot[:, c0:], v_sb[:, b * (S // 128) + kt, :],
                                     et[:, c0:], start=(kt == 0),
                                     stop=(kt == n_kt - 1))
                    ets[kt] = None
                    if kt == 0 and pending[0] is not None:
                        epilogue(pending[0])
                        pending[0] = None
                pending[0] = (rs, ot, h, q0)

            # --- interleaved emission: each chunk unblocks a set of groups ---
            # chunk c covers tokens [c*512, (c+1)*512) = batch c//2, q-block c%2
            wo_sb = None
            for c in range(NCHUNK):
                emit_chunk(c)
                b, qb = c // 2, c % 2
                for h in range(HL):
                    emit_group(b, h, qb)
                if c == NCHUNK - 1:
                    # wo reuses an hs slot (same size); DMA overlaps the
                    # final attention groups
                    wo_sb = hpool.tile([128, HL, DIM], BF16, tag="hs",
                                       name="wo_sb")
                    nc.sync.dma_start(
                        wo_sb, wo.ap().rearrange("(kh p) n -> p kh n", p=128))
            if pending[0] is not None:
                epilogue(pending[0])
                pending[0] = None

            # ---- output projection (row-parallel Wo) ----
            for tt in range(TT):
                for ni, n0 in enumerate(range(0, DIM, 512)):
                    ps = wpool.tile([128, 512], F32, tag="work", name="ps_o")
                    for kh in range(HL):
                        nc.tensor.matmul(ps, ao[:, kh, tt * 128:(tt + 1) * 128],
                                         wo_sb[:, kh, n0:n0 + 512],
                                         start=(kh == 0), stop=(kh == HL - 1))
                    osb = xsbpool.tile([128, 512], BF16, tag="osb", name="osb")
                    if (tt * 8 + ni) % 2 == 0:
                        nc.scalar.copy(osb, ps)
                    else:
                        nc.vector.tensor_copy(osb, ps)
                    nc.sync.dma_start(out.ap()[tt * 128:(tt + 1) * 128,
                                               n0:n0 + 512], osb)
    nc.compile()
    return nc


def _get_nc():
    if "nc" not in _CACHE:
        _CACHE["nc"] = _build()
    return _CACHE["nc"]


def _prep_inputs(inputs) -> list[dict]:
    bf16 = ml_dtypes.bfloat16
    hs = np.asarray(inputs["hidden_states"], dtype=np.float32).reshape(T, DIM)
    hsT = np.ascontiguousarray(hs.T).astype(bf16)

    fc = np.asarray(inputs["freqs_cos"], dtype=np.float32).reshape(T, HD // 2).T
    fs = np.asarray(inputs["freqs_sin"], dtype=np.float32).reshape(T, HD // 2).T
    cos2 = np.concatenate([fc, fc], axis=0)            # [128, T]
    sin2 = np.concatenate([-fs, fs], axis=0)           # signed half-rotation
    cos_qv = np.ascontiguousarray(cos2 * SCALE).astype(bf16)
    sin_qv = np.ascontiguousarray(sin2 * SCALE).astype(bf16)
    cos_kv = np.ascontiguousarray(cos2).astype(bf16)
    sin_kv = np.ascontiguousarray(sin2).astype(bf16)

    maskT = np.ascontiguousarray(
        np.asarray(inputs["attention_mask"], dtype=np.float32)[0, 0, :128, :128].T)

    perm = np.concatenate([np.arange(0, HD, 2), np.arange(1, HD, 2)])
    Wq = np.asarray(inputs["Wq"], dtype=np.float32)
    Wk = np.asarray(inputs["Wk"], dtype=np.float32)
    Wv = np.asarray(inputs["Wv"], dtype=np.float32)
    Wo = np.asarray(inputs["Wo"], dtype=np.float32)

    in_maps = []
    for c in range(N_CORES):
        wq_c = np.concatenate(
            [Wq[:, (c * HL + h) * HD:(c * HL + h + 1) * HD][:, perm]
             for h in range(HL)], axis=1)
        wk_c = Wk[:, c * HD:(c + 1) * HD][:, perm]
        wv_c = Wv[:, c * HD:(c + 1) * HD]
        wo_c = Wo[c * HL * HD:(c + 1) * HL * HD, :]
        in_maps.append({
            "hsT": hsT,
            "wq": np.ascontiguousarray(wq_c).astype(bf16),
            "wk": np.ascontiguousarray(wk_c).astype(bf16),
            "wv": np.ascontiguousarray(wv_c).astype(bf16),
            "wo": np.ascontiguousarray(wo_c).astype(bf16),
            "cos_q": cos_qv, "sin_q": sin_qv,
            "cos_k": cos_kv, "sin_k": sin_kv,
            "maskT": maskT,
        })
    return in_maps


def kernel(**inputs) -> np.ndarray:
    nc = _get_nc()
    in_maps = _prep_inputs(inputs)
    res = bass_utils.run_bass_kernel_spmd(nc, in_maps,
                                          core_ids=list(range(N_CORES)))
    acc = np.zeros((T, DIM), dtype=np.float32)
    for c in range(N_CORES):
        acc += np.asarray(res.results[c]["out"], dtype=np.float32)
    return acc.reshape(B, S, DIM)
